# revision 4
# baseline (speedup 1.0000x reference)
"""Trainium2 Bass kernel for nn_ModelNew_78847009620052 (dense_mlp).

Computes, for x [4096, 8192] and weight [8192, 8192]:
    out[b, 0] = 0.75 * sum_i x[b, i] * (sum_j weight[j, i])
(which equals 1.5 * sum(x @ W.T / 2, axis=1, keepdims=True)).

Sharding: column-shard the contraction dim IN=8192 into 8 chunks of 1024.
Core d receives x[:, d*1024:(d+1)*1024] and weight[:, d*1024:(d+1)*1024],
produces a partial [128, 32] result; host sums the 8 partials (after a
[128,32] -> [4096,1] reindex).

Per-core device algorithm (memory-bound: 48MB of input per core; per-NC
HBM ceiling ~358 GB/s -> ~134us DMA floor):
  Phase 1: stream weight rows as 1MB transfers ([128, 2, 1024], two
           row-tiles per DMA); tree-accumulate groups of them on VectorE,
           then accumulate the group sums on TensorE via matmul with an
           all-ones [128, 128] stationary operand - this both reduces over
           the partition (row) axis and broadcasts the column sums to all
           128 output partitions in one op. PSUM [128, 1024]. Descending
           group sizes shorten the dependency tail between the last weight
           byte and the finished column sums, which gates all of phase 2.
  Phase 2: stream 32 x row-tiles [128, 1024]; multiply against the
           broadcast column sums on VectorE (fp32, the ONLY thing VectorE
           does in phase 2 so the mul drain paces at DMA rate), reduce
           each product row-tile along the free dim on ScalarE via
           activation(Copy, accum_out=...) (1.11us/tile, 35.5us total -
           fits the 45.6us x-DMA window). The 0.75 scale is folded into
           the column sums. Results collect in an SBUF [128, 32] tile
           which is stored AS-IS (out dram tensor is [128, 32]; the host
           reindexes out[i*128+p] = O[p, i]). This kills the TensorE
           transpose + copy tail of the earlier revision.

Key streaming fixes vs the 173-182us revision:
  - xpool 12 -> 18 bufs: the x DMA issue is gated on buffer frees (mul
    completions); 18 bufs of runahead mean the gating never binds once
    VectorE is past its phase-1 backlog.
  - no DVE work besides muls in phase 2 (reduces all on ScalarE), so
    mul k issues the moment tile k lands.
(tensor_tensor_reduce would fuse phase 2 into one VectorE op, but that
opcode crashes the device on this HW/NRT path - validated by bisection.)
"""

import numpy as np

B, IN, HID = 4096, 8192, 8192
N_CORES = 8
CHUNK = IN // N_CORES          # 1024 columns per core
SCALE = 1.5 / 2.0              # 0.75
P = 128                        # partitions
W_TILES = HID // P             # 64 weight row-tiles per core
X_TILES = B // P               # 32 x row-tiles per core

_compiled_nc = None


def _build_nc():
    import concourse.bass as bass
    import concourse.tile as tile
    from concourse import bacc, mybir

    f32 = mybir.dt.float32
    nc = bacc.Bacc(
        "TRN2",
        target_bir_lowering=False,
        debug=False,
        num_devices=N_CORES,
    )

    x_d = nc.dram_tensor("x", [B, CHUNK], f32, kind="ExternalInput")
    w_d = nc.dram_tensor("w", [HID, CHUNK], f32, kind="ExternalInput")
    out_d = nc.dram_tensor("out", [P, X_TILES], f32, kind="ExternalOutput")

    with tile.TileContext(nc) as tc:
        with (
            tc.tile_pool(name="wpool", bufs=9) as wpool,
            tc.tile_pool(name="xpool", bufs=12) as xpool,
            tc.tile_pool(name="const", bufs=1) as const,
            tc.tile_pool(name="psum", bufs=1, space="PSUM") as psum_pool,
        ):
            ones = const.tile([P, P], f32)
            nc.vector.memset(ones[:], 1.0)

            # Phase 1: column sums of w chunk, reduced over all 8192 rows.
            # Each DMA moves two row-tiles (1MB) as a [128, 2, 1024] tile;
            # the first tree-add level sums the two halves in place.
            # The final TWO row-tiles go as single 512KB DMAs straight into
            # the matmul (no VectorE add): the serial tail between the last
            # weight byte and the finished column sums is then just one
            # matmul pair (~2.1us) + the broadcast copy.
            GROUPS = [4, 4, 4, 4, 4, 4, 4, 2, 1]  # in 2-row DMA units
            N_TAIL = 2                             # single row-tile DMAs
            assert sum(GROUPS) * 2 + N_TAIL == W_TILES
            psum_bc = psum_pool.tile([P, CHUNK], f32, tag="psum_bc")  # 2 banks
            row = 0
            for j, group in enumerate(GROUPS):
                wts = []
                for k in range(group):
                    wt = wpool.tile([P, 2, CHUNK], f32, tag="wtile")
                    src = w_d[(row + 2 * k) * P : (row + 2 * k + 2) * P, :]
                    nc.sync.dma_start(
                        wt[:], src.rearrange("(t p) c -> p t c", p=P)
                    )
                    nc.vector.tensor_add(
                        wt[:, 0, :], wt[:, 0, :], wt[:, 1, :]
                    )
                    wts.append(wt)
                row += 2 * group
                # tree-reduce the group accumulators in place on VectorE
                s = 1
                while s < group:
                    for k in range(0, group, 2 * s):
                        nc.vector.tensor_add(
                            wts[k][:, 0, :], wts[k][:, 0, :], wts[k + s][:, 0, :]
                        )
                    s *= 2
                for h in range(2):
                    nc.tensor.matmul(
                        psum_bc[:, h * 512 : (h + 1) * 512],
                        ones[:],
                        wts[0][:, 0, h * 512 : (h + 1) * 512],
                        start=(j == 0),
                        stop=False,
                    )
            for t in range(N_TAIL):
                wt = wpool.tile([P, CHUNK], f32, tag="wtail", bufs=2)
                nc.sync.dma_start(wt[:], w_d[row * P : (row + 1) * P, :])
                row += 1
                for h in range(2):
                    nc.tensor.matmul(
                        psum_bc[:, h * 512 : (h + 1) * 512],
                        ones[:],
                        wt[:, h * 512 : (h + 1) * 512],
                        start=False,
                        stop=(t == N_TAIL - 1),
                    )

            # Broadcast column sums now live in every PSUM partition; move to
            # SBUF on ScalarE (folding in the 0.75 scale) so VectorE stays
            # free for phase 2.
            w_bcast = const.tile([P, CHUNK], f32)
            nc.scalar.mul(w_bcast[:], psum_bc[:], SCALE)

            # Phase 2: multiply + reduce of x tiles against w_bcast.
            # Products land in PSUM (banks are free after phase 1): ScalarE's
            # PSUM-read base cost is lower than its SBUF one, and the DVE
            # PSUM-write port keeps its SBUF write port out of the way of
            # incoming x DMAs.
            s_sbuf = const.tile([P, X_TILES], f32)
            scratch = const.tile([P, CHUNK], f32)
            for i in range(X_TILES):
                xt = xpool.tile([P, CHUNK], f32, tag="xtile")
                nc.sync.dma_start(xt[:], x_d[i * P : (i + 1) * P, :])
                prod = psum_pool.tile([P, CHUNK], f32, tag="prodps", bufs=2)
                nc.vector.tensor_mul(prod[:], xt[:], w_bcast[:])
                nc.scalar.activation(
                    scratch[:],
                    prod[:],
                    mybir.ActivationFunctionType.Copy,
                    bias=0.0,
                    scale=1.0,
                    accum_out=s_sbuf[:, i : i + 1],
                )

            # Store [128, 32] as-is: 128B contiguous run per partition.
            # Host reindexes (out[i*128 + p] = O[p, i]).
            nc.sync.dma_start(out_d[:], s_sbuf[:])

    nc.compile()
    return nc


def _get_nc():
    global _compiled_nc
    if _compiled_nc is None:
        _compiled_nc = _build_nc()
    return _compiled_nc


def kernel(x: np.ndarray, weight: np.ndarray) -> np.ndarray:
    from concourse.bass_utils import run_bass_kernel_spmd

    x = np.asarray(x, dtype=np.float32)
    weight = np.asarray(weight, dtype=np.float32)
    assert x.shape == (B, IN) and weight.shape == (HID, IN)

    nc = _get_nc()
    in_maps = [
        {
            "x": np.ascontiguousarray(x[:, d * CHUNK : (d + 1) * CHUNK]),
            "w": np.ascontiguousarray(weight[:, d * CHUNK : (d + 1) * CHUNK]),
        }
        for d in range(N_CORES)
    ]
    res = run_bass_kernel_spmd(nc, in_maps, core_ids=list(range(N_CORES)))
    acc = np.zeros((B, 1), dtype=np.float64)
    for d in range(N_CORES):
        acc += res.results[d]["out"].T.reshape(B, 1).astype(np.float64)
    return acc.astype(np.float32)


# revision 6
# speedup vs baseline: 1.0066x; 1.0066x over previous
"""Trainium2 Bass kernel for nn_ModelNew_78847009620052 (dense_mlp).

Computes, for x [4096, 8192] and weight [8192, 8192]:
    out[b, 0] = 0.75 * sum_i x[b, i] * (sum_j weight[j, i])
(which equals 1.5 * sum(x @ W.T / 2, axis=1, keepdims=True)).

Sharding: column-shard the contraction dim IN=8192 into 8 chunks of 1024.
Core d receives x[:, d*1024:(d+1)*1024] and weight[:, d*1024:(d+1)*1024],
produces a partial [128, 32] result; host sums the 8 partials (after a
[128,32] -> [4096,1] reindex).

Per-core device algorithm (memory-bound: 48MB of input per core; the HW
sustains ~406 GB/s/core of HBM read when the pipeline is clean):
  Phase 1: stream weight rows; pre-accumulate on VectorE; reduce over the
           partition axis AND broadcast to 128 partitions on TensorE via
           matmul with an all-ones*0.75 stationary (scale folded in).
           Stream structure tuned for the serial tail between the last
           weight byte and the broadcast column sums being ready:
             - 2 head singles ([128,1024] 512KB DMAs): small first DMA
               so the first HBM byte lands ~1.2us earlier.
             - body of 1MB [128,2,1024] DMAs in groups [4]*7+[2] with
               VectorE tree pre-reduction (2 matmuls per group).
             - 2 tail singles, summed by ONE VectorE add, then one
               matmul pair; PSUM 'stop' on that pair.
           The PSUM->SBUF broadcast copy is split across ScalarE and
           VectorE (halves run in parallel, plain copies since the 0.75
           lives in the ones operand).
  Phase 2: stream 16 x DMAs of 1MB ([128,2,1024], two row-tiles each);
           VectorE does ONLY the products ([128,2048] per op, into SBUF);
           ScalarE reduces each [128,1024] half via
           activation(Copy, accum_out=...) into an SBUF [128,32] column.
           The [128,32] result is stored AS-IS; the host reindexes
           (out[c*128 + p] = O[p, c]).

Rationale (from NTFF traces): the kernel is pure HBM-roofline; all
engine work fits inside the DMA windows, so every revision targets the
serial tails (startup, w->x transition, post-stream drain) and keeps a
few microseconds of slack in every producer/consumer pair so one DMA
hiccup cannot re-gate the stream (the x-DMA issue is buffer-gated; with
8 x 1MB buffers the gating only engages ~6 tiles ahead of consumption).
(tensor_tensor_reduce would fuse phase 2 into one VectorE op, but that
opcode crashes the device on this HW/NRT path - validated by bisection.)
"""

import numpy as np

B, IN, HID = 4096, 8192, 8192
N_CORES = 8
CHUNK = IN // N_CORES          # 1024 columns per core
SCALE = 1.5 / 2.0              # 0.75, folded into the ones stationary
P = 128                        # partitions
W_TILES = HID // P             # 64 weight row-tiles per core
X_TILES = B // P               # 32 x row-tiles per core
XD = X_TILES // 2              # 16 x DMAs (two row-tiles each)

_compiled_nc = None


def _build_nc():
    import concourse.bass as bass
    import concourse.tile as tile
    from concourse import bacc, mybir

    f32 = mybir.dt.float32
    nc = bacc.Bacc(
        "TRN2",
        target_bir_lowering=False,
        debug=False,
        num_devices=N_CORES,
    )

    x_d = nc.dram_tensor("x", [B, CHUNK], f32, kind="ExternalInput")
    w_d = nc.dram_tensor("w", [HID, CHUNK], f32, kind="ExternalInput")
    out_d = nc.dram_tensor("out", [P, X_TILES], f32, kind="ExternalOutput")

    with tile.TileContext(nc) as tc:
        with (
            tc.tile_pool(name="wpool", bufs=9) as wpool,
            tc.tile_pool(name="xpool", bufs=8) as xpool,
            tc.tile_pool(name="const", bufs=1) as const,
            tc.tile_pool(name="psum", bufs=1, space="PSUM") as psum_pool,
        ):
            ones = const.tile([P, P], f32)
            nc.vector.memset(ones[:], SCALE)

            psum_bc = psum_pool.tile([P, CHUNK], f32, tag="psum_bc")  # 2 banks

            def colsum_pair(src_ap, start, stop):
                for h in range(2):
                    nc.tensor.matmul(
                        psum_bc[:, h * 512 : (h + 1) * 512],
                        ones[:],
                        src_ap[:, h * 512 : (h + 1) * 512],
                        start=start,
                        stop=stop,
                    )

            # --- Phase 1 head: two 512KB singles (fast first byte). ---
            row = 0
            for t in range(2):
                wt = wpool.tile([P, CHUNK], f32, tag="whead", bufs=2)
                nc.sync.dma_start(wt[:], w_d[row * P : (row + 1) * P, :])
                colsum_pair(wt, start=(t == 0), stop=False)
                row += 1

            # --- Phase 1 body: 1MB DMAs, grouped tree pre-reduction. ---
            GROUPS = [4, 4, 4, 4, 4, 4, 4, 2]  # in 2-row-tile (1MB) units
            assert 2 + sum(GROUPS) * 2 + 2 == W_TILES
            for group in GROUPS:
                wts = []
                for k in range(group):
                    wt = wpool.tile([P, 2, CHUNK], f32, tag="wtile")
                    src = w_d[(row + 2 * k) * P : (row + 2 * k + 2) * P, :]
                    nc.sync.dma_start(
                        wt[:], src.rearrange("(t p) c -> p t c", p=P)
                    )
                    nc.vector.tensor_add(
                        wt[:, 0, :], wt[:, 0, :], wt[:, 1, :]
                    )
                    wts.append(wt)
                row += 2 * group
                s = 1
                while s < group:
                    for k in range(0, group, 2 * s):
                        nc.vector.tensor_add(
                            wts[k][:, 0, :], wts[k][:, 0, :], wts[k + s][:, 0, :]
                        )
                    s *= 2
                colsum_pair(wts[0][:, 0, :], start=False, stop=False)

            # --- Phase 1 tail: two 512KB singles, one VectorE add, one
            # matmul pair carrying the PSUM stop. ---
            t0 = wpool.tile([P, CHUNK], f32, tag="wtail", bufs=2)
            nc.sync.dma_start(t0[:], w_d[row * P : (row + 1) * P, :])
            t1 = wpool.tile([P, CHUNK], f32, tag="wtail", bufs=2)
            nc.sync.dma_start(t1[:], w_d[(row + 1) * P : (row + 2) * P, :])
            row += 2
            assert row == W_TILES
            nc.vector.tensor_add(t0[:], t0[:], t1[:])
            colsum_pair(t0, start=False, stop=True)

            # Broadcast column sums PSUM -> SBUF, halves in parallel on
            # ScalarE + VectorE (both plain copies; 0.75 already applied).
            # w_bcast2 holds the sums twice so phase-2 muls run [128,2048].
            w_bcast2 = const.tile([P, 2, CHUNK], f32)
            nc.scalar.copy(w_bcast2[:, 0, :], psum_bc[:])
            nc.vector.tensor_copy(w_bcast2[:, 1, :], psum_bc[:])

            # --- Phase 2: x stream, products on VectorE, reduces on
            # ScalarE. ---
            s_sbuf = const.tile([P, X_TILES], f32)
            scratch = const.tile([P, CHUNK], f32)
            wb_flat = w_bcast2[:].rearrange("p t c -> p (t c)")
            for i in range(XD):
                xt = xpool.tile([P, 2, CHUNK], f32, tag="xtile")
                src = x_d[2 * i * P : (2 * i + 2) * P, :]
                nc.sync.dma_start(xt[:], src.rearrange("(t p) c -> p t c", p=P))
                prod = xpool.tile([P, 2, CHUNK], f32, tag="prod", bufs=2)
                nc.vector.tensor_mul(
                    prod[:].rearrange("p t c -> p (t c)"),
                    xt[:].rearrange("p t c -> p (t c)"),
                    wb_flat,
                )
                for h in range(2):
                    nc.scalar.activation(
                        scratch[:],
                        prod[:, h, :],
                        mybir.ActivationFunctionType.Copy,
                        bias=0.0,
                        scale=1.0,
                        accum_out=s_sbuf[:, 2 * i + h : 2 * i + h + 1],
                    )

            # Store [128, 32] as-is: 128B contiguous run per partition.
            nc.sync.dma_start(out_d[:], s_sbuf[:])

    nc.compile()
    return nc


def _get_nc():
    global _compiled_nc
    if _compiled_nc is None:
        _compiled_nc = _build_nc()
    return _compiled_nc


def kernel(x: np.ndarray, weight: np.ndarray) -> np.ndarray:
    from concourse.bass_utils import run_bass_kernel_spmd

    x = np.asarray(x, dtype=np.float32)
    weight = np.asarray(weight, dtype=np.float32)
    assert x.shape == (B, IN) and weight.shape == (HID, IN)

    nc = _get_nc()
    in_maps = [
        {
            "x": np.ascontiguousarray(x[:, d * CHUNK : (d + 1) * CHUNK]),
            "w": np.ascontiguousarray(weight[:, d * CHUNK : (d + 1) * CHUNK]),
        }
        for d in range(N_CORES)
    ]
    res = run_bass_kernel_spmd(nc, in_maps, core_ids=list(range(N_CORES)))
    acc = np.zeros((B, 1), dtype=np.float64)
    for d in range(N_CORES):
        acc += res.results[d]["out"].T.reshape(B, 1).astype(np.float64)
    return acc.astype(np.float32)


# revision 10
# speedup vs baseline: 1.0124x; 1.0057x over previous
"""Trainium2 Bass kernel for nn_ModelNew_78847009620052 (dense_mlp).

Computes, for x [4096, 8192] and weight [8192, 8192]:
    out[b, 0] = 0.75 * sum_i x[b, i] * (sum_j weight[j, i])
(which equals 1.5 * sum(x @ W.T / 2, axis=1, keepdims=True)).

Sharding: column-shard the contraction dim IN=8192 into 8 chunks of 1024.
Core d receives x[:, d*1024:(d+1)*1024] and weight[:, d*1024:(d+1)*1024],
produces a partial [128, 32] result; host sums the 8 partials (after a
[128,32] -> [4096,1] reindex).

Per-core device algorithm (memory-bound: 48MB of input per core; the HW
sustains ~406 GB/s/core of HBM read when the pipeline is clean):
  Phase 1: stream weight rows; pre-accumulate on VectorE; reduce over the
           partition axis AND broadcast to 128 partitions on TensorE via
           matmul with an all-ones*0.75 stationary (scale folded in).
           Stream structure tuned for the serial tail between the last
           weight byte and the broadcast column sums being ready:
             - 2 head singles ([128,1024] 512KB DMAs): small first DMA
               so the first HBM byte lands ~1.2us earlier.
             - body of 1MB [128,2,1024] DMAs in groups [4]*7+[2] with
               VectorE tree pre-reduction (2 matmuls per group).
             - 2 tail singles, summed by ONE VectorE add, then one
               matmul pair; PSUM 'stop' on that pair.
           The PSUM->SBUF broadcast copy is split across ScalarE and
           VectorE (halves run in parallel, plain copies since the 0.75
           lives in the ones operand).
  Phase 2: stream 16 x DMAs of 1MB ([128,2,1024], two row-tiles each);
           VectorE does ONLY the products ([128,2048] per op, into SBUF);
           ScalarE reduces each [128,1024] half via
           activation(Copy, accum_out=...) into an SBUF [128,32] column.
           The [128,32] result is stored AS-IS; the host reindexes
           (out[c*128 + p] = O[p, c]).

Rationale (from NTFF traces): the kernel is pure HBM-roofline; all
engine work fits inside the DMA windows, so every revision targets the
serial tails (startup, w->x transition, post-stream drain) and keeps a
few microseconds of slack in every producer/consumer pair so one DMA
hiccup cannot re-gate the stream (the x-DMA issue is buffer-gated; with
8 x 1MB buffers the gating only engages ~6 tiles ahead of consumption).
(tensor_tensor_reduce would fuse phase 2 into one VectorE op, but that
opcode crashes the device on this HW/NRT path - validated by bisection.)
"""

import numpy as np

B, IN, HID = 4096, 8192, 8192
N_CORES = 8
CHUNK = IN // N_CORES          # 1024 columns per core
SCALE = 1.5 / 2.0              # 0.75, folded into the ones stationary
P = 128                        # partitions
W_TILES = HID // P             # 64 weight row-tiles per core
X_TILES = B // P               # 32 x row-tiles per core
XD = X_TILES // 2              # 16 x DMAs (two row-tiles each)

_compiled_nc = None


def _build_nc():
    import concourse.bass as bass
    import concourse.tile as tile
    from concourse import bacc, mybir

    f32 = mybir.dt.float32
    nc = bacc.Bacc(
        "TRN2",
        target_bir_lowering=False,
        debug=False,
        num_devices=N_CORES,
    )

    x_d = nc.dram_tensor("x", [B, CHUNK], f32, kind="ExternalInput")
    w_d = nc.dram_tensor("w", [HID, CHUNK], f32, kind="ExternalInput")
    out_d = nc.dram_tensor("out", [P, X_TILES], f32, kind="ExternalOutput")

    with tile.TileContext(nc) as tc:
        with (
            tc.tile_pool(name="wpool", bufs=9) as wpool,
            tc.tile_pool(name="xpool", bufs=6) as xpool,
            tc.tile_pool(name="const", bufs=1) as const,
            tc.tile_pool(name="psum", bufs=1, space="PSUM") as psum_pool,
        ):
            ones = const.tile([P, P], f32)
            nc.vector.memset(ones[:], SCALE)

            # Column sums land TWICE in PSUM ([P, 2, CHUNK], 4 banks) so
            # phase-2 muls can read a [128, 2048] broadcast operand straight
            # from PSUM - no PSUM->SBUF copy on the critical transition.
            psum_bc = psum_pool.tile([P, 2, CHUNK], f32, tag="psum_bc")

            def colsum_pair(src_ap, start, stop):
                for r in range(2):
                    for h in range(2):
                        nc.tensor.matmul(
                            psum_bc[:, r, h * 512 : (h + 1) * 512],
                            ones[:],
                            src_ap[:, h * 512 : (h + 1) * 512],
                            start=start,
                            stop=stop,
                        )

            # --- Phase 1 head: two 512KB singles (fast first byte). ---
            row = 0
            for t in range(2):
                wt = wpool.tile([P, CHUNK], f32, tag="whead", bufs=2)
                nc.sync.dma_start(wt[:], w_d[row * P : (row + 1) * P, :])
                colsum_pair(wt, start=(t == 0), stop=False)
                row += 1

            # --- Phase 1 body: 1MB DMAs, grouped tree pre-reduction.
            # The trailing [1, 1] groups keep the TensorE queue clear when
            # the tail singles land (their matmul pair is the only PSUM
            # work left). ---
            GROUPS = [4, 4, 4, 4, 4, 4, 4, 1, 1]  # in 2-row-tile (1MB) units
            assert 2 + sum(GROUPS) * 2 + 2 == W_TILES
            for group in GROUPS:
                wts = []
                for k in range(group):
                    wt = wpool.tile([P, 2, CHUNK], f32, tag="wtile")
                    src = w_d[(row + 2 * k) * P : (row + 2 * k + 2) * P, :]
                    nc.sync.dma_start(
                        wt[:], src.rearrange("(t p) c -> p t c", p=P)
                    )
                    nc.vector.tensor_add(
                        wt[:, 0, :], wt[:, 0, :], wt[:, 1, :]
                    )
                    wts.append(wt)
                row += 2 * group
                s = 1
                while s < group:
                    for k in range(0, group, 2 * s):
                        nc.vector.tensor_add(
                            wts[k][:, 0, :], wts[k][:, 0, :], wts[k + s][:, 0, :]
                        )
                    s *= 2
                colsum_pair(wts[0][:, 0, :], start=False, stop=False)

            # --- Phase 1 tail: two 512KB singles, one VectorE add, one
            # matmul pair carrying the PSUM stop. ---
            t0 = wpool.tile([P, CHUNK], f32, tag="wtail", bufs=2)
            nc.sync.dma_start(t0[:], w_d[row * P : (row + 1) * P, :])
            t1 = wpool.tile([P, CHUNK], f32, tag="wtail", bufs=2)
            nc.sync.dma_start(t1[:], w_d[(row + 1) * P : (row + 2) * P, :])
            row += 2
            assert row == W_TILES
            nc.vector.tensor_add(t0[:], t0[:], t1[:])
            colsum_pair(t0, start=False, stop=True)

            # --- Phase 2: x stream, fused [128,2048] products on VectorE
            # reading the broadcast sums straight from PSUM, reduces on
            # ScalarE from SBUF. ---
            s_sbuf = const.tile([P, X_TILES], f32)
            scratch = const.tile([P, CHUNK], f32)
            wb_flat = psum_bc[:].rearrange("p t c -> p (t c)")
            for i in range(XD):
                xt = xpool.tile([P, 2, CHUNK], f32, tag="xtile")
                src = x_d[2 * i * P : (2 * i + 2) * P, :]
                nc.sync.dma_start(xt[:], src.rearrange("(t p) c -> p t c", p=P))
                prod = xpool.tile([P, 2, CHUNK], f32, tag="prod", bufs=3)
                nc.vector.tensor_mul(
                    prod[:].rearrange("p t c -> p (t c)"),
                    xt[:].rearrange("p t c -> p (t c)"),
                    wb_flat,
                )
                for h in range(2):
                    nc.scalar.activation(
                        scratch[:],
                        prod[:, h, :],
                        mybir.ActivationFunctionType.Copy,
                        bias=0.0,
                        scale=1.0,
                        accum_out=s_sbuf[:, 2 * i + h : 2 * i + h + 1],
                    )

            # Store [128, 32] as-is: 128B contiguous run per partition.
            nc.sync.dma_start(out_d[:], s_sbuf[:])

    nc.compile()
    return nc


def _get_nc():
    global _compiled_nc
    if _compiled_nc is None:
        _compiled_nc = _build_nc()
    return _compiled_nc


def kernel(x: np.ndarray, weight: np.ndarray) -> np.ndarray:
    from concourse.bass_utils import run_bass_kernel_spmd

    x = np.asarray(x, dtype=np.float32)
    weight = np.asarray(weight, dtype=np.float32)
    assert x.shape == (B, IN) and weight.shape == (HID, IN)

    nc = _get_nc()
    in_maps = [
        {
            "x": np.ascontiguousarray(x[:, d * CHUNK : (d + 1) * CHUNK]),
            "w": np.ascontiguousarray(weight[:, d * CHUNK : (d + 1) * CHUNK]),
        }
        for d in range(N_CORES)
    ]
    res = run_bass_kernel_spmd(nc, in_maps, core_ids=list(range(N_CORES)))
    acc = np.zeros((B, 1), dtype=np.float64)
    for d in range(N_CORES):
        acc += res.results[d]["out"].T.reshape(B, 1).astype(np.float64)
    return acc.astype(np.float32)


# revision 12
# speedup vs baseline: 1.0408x; 1.0280x over previous
"""Trainium2 Bass kernel for nn_ModelNew_78847009620052 (dense_mlp).

Computes, for x [4096, 8192] and weight [8192, 8192]:
    out[b, 0] = 0.75 * sum_i x[b, i] * (sum_j weight[j, i])
(which equals 1.5 * sum(x @ W.T / 2, axis=1, keepdims=True)).

Sharding: column-shard the contraction dim IN=8192 into 8 chunks of 1024.
Core d receives x[:, d*1024:(d+1)*1024] and weight[:, d*1024:(d+1)*1024],
produces a partial [128, 32] result; host sums the 8 partials (after a
[128,32] -> [4096,1] reindex).

Per-core device algorithm (memory-bound: 48MB of input per core; the HW
sustains ~406 GB/s/core of HBM read when the pipeline is clean):
  Phase 1: stream weight rows; pre-accumulate on VectorE; reduce over the
           partition axis AND broadcast to 128 partitions on TensorE via
           matmul with an all-ones*0.75 stationary (scale folded in).
           Stream structure tuned for the serial tail between the last
           weight byte and the broadcast column sums being ready:
             - 2 head singles ([128,1024] 512KB DMAs): small first DMA
               so the first HBM byte lands ~1.2us earlier.
             - body of 1MB [128,2,1024] DMAs in groups [4]*7+[2] with
               VectorE tree pre-reduction (2 matmuls per group).
             - 2 tail singles, summed by ONE VectorE add, then one
               matmul pair; PSUM 'stop' on that pair.
           The PSUM->SBUF broadcast copy is split across ScalarE and
           VectorE (halves run in parallel, plain copies since the 0.75
           lives in the ones operand).
  Phase 2: stream 16 x DMAs of 1MB ([128,2,1024], two row-tiles each);
           VectorE does ONLY the products ([128,2048] per op, into SBUF);
           ScalarE reduces each [128,1024] half via
           activation(Copy, accum_out=...) into an SBUF [128,32] column.
           The [128,32] result is stored AS-IS; the host reindexes
           (out[c*128 + p] = O[p, c]).

Rationale (from NTFF traces): the kernel is pure HBM-roofline; all
engine work fits inside the DMA windows, so every revision targets the
serial tails (startup, w->x transition, post-stream drain) and keeps a
few microseconds of slack in every producer/consumer pair so one DMA
hiccup cannot re-gate the stream (the x-DMA issue is buffer-gated; with
8 x 1MB buffers the gating only engages ~6 tiles ahead of consumption).
(tensor_tensor_reduce would fuse phase 2 into one VectorE op, but that
opcode crashes the device on this HW/NRT path - validated by bisection.)
"""

import numpy as np

B, IN, HID = 4096, 8192, 8192
N_CORES = 8
CHUNK = IN // N_CORES          # 1024 columns per core
SCALE = 1.5 / 2.0              # 0.75, folded into the ones stationary
P = 128                        # partitions
W_TILES = HID // P             # 64 weight row-tiles per core
X_TILES = B // P               # 32 x row-tiles per core
XD = X_TILES // 2              # 16 x DMAs (two row-tiles each)

_compiled_nc = None


def _build_nc():
    import concourse.bass as bass
    import concourse.tile as tile
    from concourse import bacc, mybir

    f32 = mybir.dt.float32
    nc = bacc.Bacc(
        "TRN2",
        target_bir_lowering=False,
        debug=False,
        num_devices=N_CORES,
    )

    x_d = nc.dram_tensor("x", [B, CHUNK], f32, kind="ExternalInput")
    w_d = nc.dram_tensor("w", [HID, CHUNK], f32, kind="ExternalInput")
    out_d = nc.dram_tensor("out", [P, X_TILES], f32, kind="ExternalOutput")

    with tile.TileContext(nc) as tc:
        with (
            tc.tile_pool(name="wpool", bufs=9) as wpool,
            tc.tile_pool(name="xpool", bufs=6) as xpool,
            tc.tile_pool(name="const", bufs=1) as const,
            tc.tile_pool(name="psum", bufs=1, space="PSUM") as psum_pool,
        ):
            ones = const.tile([P, P], f32)
            nc.vector.memset(ones[:], SCALE)

            # Column sums land TWICE in PSUM ([P, 2, CHUNK], 4 banks) so
            # phase-2 muls can read a [128, 2048] broadcast operand straight
            # from PSUM - no PSUM->SBUF copy on the critical transition.
            psum_bc = psum_pool.tile([P, 2, CHUNK], f32, tag="psum_bc")

            def colsum_pair(src_ap, start, stop):
                for h in range(2):
                    nc.tensor.matmul(
                        psum_bc[:, 0, h * 512 : (h + 1) * 512],
                        ones[:],
                        src_ap[:, h * 512 : (h + 1) * 512],
                        start=start,
                        stop=stop,
                    )

            # --- Phase 1 head: two 512KB singles (fast first byte). ---
            row = 0
            for t in range(2):
                wt = wpool.tile([P, CHUNK], f32, tag="whead", bufs=2)
                nc.sync.dma_start(wt[:], w_d[row * P : (row + 1) * P, :])
                colsum_pair(wt, start=(t == 0), stop=False)
                row += 1

            # --- Phase 1 body: 1MB DMAs, grouped tree pre-reduction.
            # The trailing [1, 1] groups keep the TensorE queue clear when
            # the tail singles land (their matmul pair is the only PSUM
            # work left). ---
            GROUPS = [4, 4, 4, 4, 4, 4, 4, 1, 1]  # in 2-row-tile (1MB) units
            assert 2 + sum(GROUPS) * 2 + 2 == W_TILES
            for group in GROUPS:
                wts = []
                for k in range(group):
                    wt = wpool.tile([P, 2, CHUNK], f32, tag="wtile")
                    src = w_d[(row + 2 * k) * P : (row + 2 * k + 2) * P, :]
                    nc.sync.dma_start(
                        wt[:], src.rearrange("(t p) c -> p t c", p=P)
                    )
                    nc.vector.tensor_add(
                        wt[:, 0, :], wt[:, 0, :], wt[:, 1, :]
                    )
                    wts.append(wt)
                row += 2 * group
                s = 1
                while s < group:
                    for k in range(0, group, 2 * s):
                        nc.vector.tensor_add(
                            wts[k][:, 0, :], wts[k][:, 0, :], wts[k + s][:, 0, :]
                        )
                    s *= 2
                colsum_pair(wts[0][:, 0, :], start=False, stop=False)

            # --- Phase 1 tail: two 512KB singles, one VectorE add, one
            # matmul pair carrying the PSUM stop. ---
            t0 = wpool.tile([P, CHUNK], f32, tag="wtail", bufs=2)
            nc.sync.dma_start(t0[:], w_d[row * P : (row + 1) * P, :])
            t1 = wpool.tile([P, CHUNK], f32, tag="wtail", bufs=2)
            nc.sync.dma_start(t1[:], w_d[(row + 1) * P : (row + 2) * P, :])
            row += 2
            assert row == W_TILES
            nc.vector.tensor_add(t0[:], t0[:], t1[:])
            colsum_pair(t0, start=False, stop=True)
            # Duplicate the finished column sums into region 1 (one DVE
            # PSUM->PSUM copy) so the muls get a [128, 2048] operand.
            nc.vector.tensor_copy(psum_bc[:, 1, :], psum_bc[:, 0, :])

            # --- Phase 2: x stream, fused [128,2048] products on VectorE
            # reading the broadcast sums straight from PSUM, reduces on
            # ScalarE from SBUF. ---
            s_sbuf = const.tile([P, X_TILES], f32)
            scratch = const.tile([P, CHUNK], f32)
            wb_flat = psum_bc[:].rearrange("p t c -> p (t c)")
            for i in range(XD):
                xt = xpool.tile([P, 2, CHUNK], f32, tag="xtile")
                src = x_d[2 * i * P : (2 * i + 2) * P, :]
                nc.sync.dma_start(xt[:], src.rearrange("(t p) c -> p t c", p=P))
                prod = xpool.tile([P, 2, CHUNK], f32, tag="prod", bufs=3)
                nc.vector.tensor_mul(
                    prod[:].rearrange("p t c -> p (t c)"),
                    xt[:].rearrange("p t c -> p (t c)"),
                    wb_flat,
                )
                for h in range(2):
                    nc.scalar.activation(
                        scratch[:],
                        prod[:, h, :],
                        mybir.ActivationFunctionType.Copy,
                        bias=0.0,
                        scale=1.0,
                        accum_out=s_sbuf[:, 2 * i + h : 2 * i + h + 1],
                    )

            # Store [128, 32] as-is: 128B contiguous run per partition.
            nc.sync.dma_start(out_d[:], s_sbuf[:])

    nc.compile()
    return nc


def _get_nc():
    global _compiled_nc
    if _compiled_nc is None:
        _compiled_nc = _build_nc()
    return _compiled_nc


def kernel(x: np.ndarray, weight: np.ndarray) -> np.ndarray:
    from concourse.bass_utils import run_bass_kernel_spmd

    x = np.asarray(x, dtype=np.float32)
    weight = np.asarray(weight, dtype=np.float32)
    assert x.shape == (B, IN) and weight.shape == (HID, IN)

    nc = _get_nc()
    in_maps = [
        {
            "x": np.ascontiguousarray(x[:, d * CHUNK : (d + 1) * CHUNK]),
            "w": np.ascontiguousarray(weight[:, d * CHUNK : (d + 1) * CHUNK]),
        }
        for d in range(N_CORES)
    ]
    res = run_bass_kernel_spmd(nc, in_maps, core_ids=list(range(N_CORES)))
    acc = np.zeros((B, 1), dtype=np.float64)
    for d in range(N_CORES):
        acc += res.results[d]["out"].T.reshape(B, 1).astype(np.float64)
    return acc.astype(np.float32)
